# revision 1
# baseline (speedup 1.0000x reference)
"""Expert-parallel MoE FFN kernel for Trainium2 (Bass/Tile).

Problem: y[b,e,n,:] = gelu(x[b,e,n,:] @ w1[e] + b1[e]) @ w2[e] + b2[e]
Shapes:  x (2,8,2048,1024), w1 (8,1024,4096), b1 (8,4096),
         w2 (8,4096,1024), b2 (8,1024)  -> out (2,8,2048,1024) fp32.

Sharding: expert-parallel, one expert per NeuronCore (8 cores).  Each core
processes its expert's 4096 tokens through the full FFN locally; no
cross-core communication.

Per-core dataflow (all matmuls in float32r at N=512 -> full PE rate):
  Phase 1:  xT = transpose(x_e)  (PE transpose, 128x128 tiles)
            hT[h,t] = gelu(sum_d w1[d,h] * xT[d,t] + b1[h])   (hT: [H,T])
            hT staged to DRAM (doesn't fit SBUF alongside weights).
  Phase 2:  y[t,d] = sum_h hT[h,t] * w2[h,d] (+ b2)
The hT (activation-transposed) layout means the big [H,T] intermediate is
produced and consumed with no transposes; only x needs a transpose on the
way in, and y comes out in natural [T,D] layout.

Scheduling notes:
 - GEMM1 runs group-outer (token-chunk groups) so the first-half xT tile can
   be freed at the GEMM1 midpoint; the leading slice of w2 loads into that
   space, hiding most of the phase-2 weight-load latency.
 - PSUM: transposes use 6 banks (their pool closes before GEMM1); GEMM1 uses
   [128, 4*512] psum tiles double-buffered (all 8 banks) so the GELU drain of
   one group overlaps the next group's matmuls; phase 2 reuses the freed
   banks for [128, 1024] x2.
 - w1 streams in [128, 8, 256] chunks (1 KB rows); w2 preloads 24 of 32
   h-tiles during the second GEMM1 half, the rest right after the first
   phase-2 hT load wins the DMA-queue race.
"""

import numpy as np
from contextlib import ExitStack

import concourse.bass as bass
import concourse.mybir as mybir
import concourse.tile as tile
from concourse import bacc
from concourse.bass_utils import run_bass_kernel_spmd
from concourse.masks import make_identity

P = 128
F32 = mybir.dt.float32
F32R = mybir.dt.float32r

# Full-size problem constants (hardcoded; the grading harness calls
# kernel(**inputs) with exactly these shapes).
B, E, N, D, H = 2, 8, 2048, 1024, 4096
N_CORES = 8


def emit_expert_ffn(tc, x, w1, b1, w2, b2, y, hT_st, T, D_, H_, TCH=512, DCH=512,
                    G=4, use_b2=False):
    """Emit one expert's FFN. x:[T,D] w1:[D,H] b1:[H] w2:[H,D] b2:[D] y:[T,D].

    hT_st: [H, T] DRAM staging tile (float32r) for the transposed activation.
    TCH: token chunk (matmul moving free dim) for GEMM1.
    DCH: output-dim chunk for GEMM2 (one PSUM bank).
    G:   token chunks per PSUM accumulation group in GEMM1.
    """
    nc = tc.nc
    NT = T // P          # token subtiles
    ND = D_ // P         # contraction tiles for GEMM1
    NH = H_ // P         # h tiles
    NCH = T // TCH       # token chunks
    NG = NCH // G        # chunk groups
    NDC = D_ // DCH      # output chunks for GEMM2
    NG_A = NG // 2       # groups reading the first-half xT tile
    W2A = min(24, NH)    # leading w2 h-tiles loaded during gemm1b

    assert T % P == 0 and D_ % P == 0 and H_ % P == 0
    assert T % TCH == 0 and NCH % G == 0 and D_ % DCH == 0 and NG % 2 == 0
    T_half = NG_A * G * TCH

    hT_r = hT_st.rearrange("(ht p) t -> p ht t", p=P)
    w2_r = w2.rearrange("(ht p) d -> p ht d", p=P)

    with ExitStack() as es:
        const_pool = es.enter_context(tc.tile_pool(name="const", bufs=1, side="right"))
        ident = const_pool.tile([P, P], F32)
        make_identity(nc, ident)
        b1_sb = const_pool.tile([P, NH], F32)
        nc.sync.dma_start(b1_sb[:], b1.rearrange("(ht p) -> p ht", p=P))
        if use_b2:
            b2_sb = const_pool.tile([P, D_], F32)
            nc.sync.dma_start(b2_sb[:], b2.unsqueeze(0).broadcast_to([P, D_]))

        # -------- Phase 1: xT transpose + hT = gelu(w1.T @ xT + b1) --------
        p1 = ExitStack()
        w1_pool = p1.enter_context(tc.tile_pool(name="w1_pool", bufs=2, side="left"))
        hout_pool = p1.enter_context(tc.tile_pool(name="hout_pool", bufs=2, side="left"))
        xTb_pool = p1.enter_context(tc.tile_pool(name="xTb_pool", bufs=1, side="left"))
        xTa_es = ExitStack()
        xTa_pool = xTa_es.enter_context(tc.tile_pool(name="xTa_pool", bufs=1, side="left"))

        xT_a = xTa_pool.tile([P, ND, T_half], F32R)
        xT_b = xTb_pool.tile([P, ND, T - T_half], F32R)

        def xT_slice(dt, t0, t1):
            if t1 <= T_half:
                return xT_a[:, dt, t0:t1]
            assert t0 >= T_half
            return xT_b[:, dt, t0 - T_half:t1 - T_half]

        with nc.named_scope("transpose"):
            with (
                tc.tile_pool(name="xraw_pool", bufs=5, side="left") as xraw_pool,
                tc.tile_pool(name="ptp_pool", bufs=6, space="PSUM",
                             side="left") as ptp_pool,
            ):
                for tsub in range(NT):
                    x_raw = xraw_pool.tile([P, D_], F32, name="x_raw")
                    DQ = max(P, D_ // 4)
                    for q0 in range(0, D_, DQ):
                        nc.sync.dma_start(
                            x_raw[:, q0:q0 + DQ],
                            x[tsub * P:(tsub + 1) * P, q0:q0 + DQ])
                    for dt in range(ND):
                        ptp = ptp_pool.tile([P, P], F32, name="ptp")
                        nc.tensor.transpose(ptp[:],
                                            x_raw[:, dt * P:(dt + 1) * P],
                                            ident[:])
                        nc.vector.tensor_copy(
                            xT_slice(dt, tsub * P, (tsub + 1) * P), ptp[:])

        # GEMM1 psum: [128, G*512] = 4 banks, double buffered = all 8 banks
        # (transpose psum pool closed above).
        ph_pool = p1.enter_context(tc.tile_pool(name="ph_pool", bufs=2,
                                                space="PSUM", side="left"))

        HTG = 2              # h-tiles per w1 chunk load (1KB DMA rows)
        assert NH % HTG == 0

        def gemm1_group(g, drip=None):
            for htg in range(NH // HTG):
                if drip:
                    for _ in range(3):
                        if drip:
                            drip.pop(0)()
                w1_t = w1_pool.tile([P, ND, HTG * P], F32R, name="w1_t")
                for dt in range(ND):
                    nc.sync.dma_start(
                        w1_t[:, dt, :],
                        w1[dt * P:(dt + 1) * P,
                           htg * HTG * P:(htg + 1) * HTG * P])
                for hl in range(HTG):
                    ht = htg * HTG + hl
                    psum_h = ph_pool.tile([P, G * TCH], F32, name="psum_h")
                    for dt in range(ND):
                        for i in range(G):
                            tc0 = (g * G + i) * TCH
                            nc.tensor.matmul(
                                psum_h[:, i * TCH:(i + 1) * TCH],
                                w1_t[:, dt, hl * P:(hl + 1) * P],
                                xT_slice(dt, tc0, tc0 + TCH),
                                start=(dt == 0), stop=(dt == ND - 1))
                    hT_out = hout_pool.tile([P, G * TCH], F32R, name="hT_out")
                    for i in range(G):
                        nc.scalar.activation(
                            hT_out[:, i * TCH:(i + 1) * TCH],
                            psum_h[:, i * TCH:(i + 1) * TCH],
                            mybir.ActivationFunctionType.Gelu_apprx_tanh,
                            bias=b1_sb[:, ht:ht + 1], scale=1.0)
                    nc.sync.dma_start(
                        hT_st[ht * P:(ht + 1) * P,
                              g * G * TCH:(g + 1) * G * TCH],
                        hT_out[:])

        with nc.named_scope("gemm1a"):
            for g in range(NG_A):
                gemm1_group(g)
        # first-half xT is dead; free its SBUF for the leading w2 slice.
        xTa_es.close()

        p2 = ExitStack()
        w2a_pool = p2.enter_context(tc.tile_pool(name="w2a_pool", bufs=1, side="right"))
        w2a = w2a_pool.tile([P, W2A, D_], F32R)

        def _w2a_load(ht):
            return lambda: nc.sync.dma_start(w2a[:, ht, :], w2_r[:, ht, :])

        drip = [_w2a_load(ht) for ht in range(W2A)]
        with nc.named_scope("gemm1b"):
            for g in range(NG_A, NG):
                gemm1_group(g, drip=drip)
        for thunk in drip:
            thunk()
        p1.close()

        # -------- Phase 2: y = hT.T @ w2 (+ b2) ----------------------------
        w2b_pool = p2.enter_context(tc.tile_pool(name="w2b_pool", bufs=1, side="right"))
        hTin_pool = p2.enter_context(tc.tile_pool(name="hTin_pool", bufs=3, side="right"))
        out_pool = p2.enter_context(tc.tile_pool(name="out_pool", bufs=3, side="right"))
        po_pool = p2.enter_context(tc.tile_pool(name="po_pool", bufs=2,
                                                space="PSUM", side="right"))
        w2b = (w2b_pool.tile([P, NH - W2A, D_], F32R, name="w2b")
               if NH > W2A else None)

        def w2_sb(ht):
            return w2a[:, ht, :] if ht < W2A else w2b[:, ht - W2A, :]

        with nc.named_scope("gemm2"):
            for tt in range(NT):
                hT_in = hTin_pool.tile([P, NH, P], F32R, name="hT_in")
                for hq in range(0, NH, NH // 4):
                    nc.sync.dma_start(
                        hT_in[:, hq:hq + NH // 4, :],
                        hT_r[:, hq:hq + NH // 4, tt * P:(tt + 1) * P])
                if tt == 0 and w2b is not None:
                    # emitted after the first hT load so those DMAs win the
                    # queue race; needed from the ht=W2A matmul onward.
                    for ht in range(W2A, NH):
                        nc.sync.dma_start(w2b[:, ht - W2A, :], w2_r[:, ht, :])
                psum_o = po_pool.tile([P, D_], F32, name="psum_o")
                for ht in range(NH):
                    for dc in range(NDC):
                        nc.tensor.matmul(
                            psum_o[:, dc * DCH:(dc + 1) * DCH],
                            hT_in[:, ht, :],
                            w2_sb(ht)[:, dc * DCH:(dc + 1) * DCH],
                            start=(ht == 0), stop=(ht == NH - 1))
                out_sb = out_pool.tile([P, D_], F32, name="out_sb")
                for dc in range(NDC):
                    sl = slice(dc * DCH, (dc + 1) * DCH)
                    if use_b2:
                        nc.vector.tensor_add(out_sb[:, sl], psum_o[:, sl],
                                             b2_sb[:, sl])
                    else:
                        nc.scalar.copy(out_sb[:, sl], psum_o[:, sl])
                    nc.sync.dma_start(y[tt * P:(tt + 1) * P, sl],
                                      out_sb[:, sl])
        p2.close()


def build_module(T, D_, H_, TCH=512, DCH=512, use_b2=False):
    nc = bacc.Bacc(None, target_bir_lowering=False)
    x = nc.dram_tensor("x", [T, D_], F32, kind="ExternalInput")
    w1 = nc.dram_tensor("w1", [D_, H_], F32R, kind="ExternalInput")
    b1 = nc.dram_tensor("b1", [H_], F32, kind="ExternalInput")
    w2 = nc.dram_tensor("w2", [H_, D_], F32R, kind="ExternalInput")
    if use_b2:
        b2 = nc.dram_tensor("b2", [D_], F32, kind="ExternalInput")
    else:
        b2 = None
    y = nc.dram_tensor("y", [T, D_], F32, kind="ExternalOutput")

    with tile.TileContext(nc) as tc:
        with tc.tile_pool(name="dram_st", bufs=1, space="DRAM") as dram_pool:
            hT_st = dram_pool.tile([H_, T], F32R)
            emit_expert_ffn(tc, x[:], w1[:], b1[:], w2[:],
                            b2[:] if use_b2 else None, y[:], hT_st,
                            T, D_, H_, TCH=TCH, DCH=DCH, use_b2=use_b2)
    nc.compile()
    return nc


_module_cache = {}


def _get_module(key):
    if key not in _module_cache:
        T, D_, H_, use_b2 = key
        _module_cache[key] = build_module(T, D_, H_, use_b2=use_b2)
    return _module_cache[key]


def run_moe(x, w1, b1, w2, b2, trace=False):
    """x:(B,E,N,D) w1:(E,D,H) b1:(E,H) w2:(E,H,D) b2:(E,D) -> (B,E,N,D)."""
    Bx, Ex, Nx, Dx = x.shape
    Hx = w1.shape[2]
    T = Bx * Nx
    use_b2 = bool(np.any(b2))
    nc = _get_module((T, Dx, Hx, use_b2))

    in_maps = []
    for e in range(Ex):
        m = {
            "x": np.ascontiguousarray(x[:, e]).reshape(T, Dx),
            "w1": np.ascontiguousarray(w1[e]),
            "b1": np.ascontiguousarray(b1[e]),
            "w2": np.ascontiguousarray(w2[e]),
        }
        if use_b2:
            m["b2"] = np.ascontiguousarray(b2[e])
        in_maps.append(m)

    br = run_bass_kernel_spmd(nc, in_maps, core_ids=list(range(Ex)),
                              trace=trace)
    ys = np.stack([br.results[e]["y"] for e in range(Ex)], axis=0)  # [E,T,D]
    out = ys.reshape(Ex, Bx, Nx, Dx).reshape(Bx, Ex, Nx, Dx)
    return (out, br) if trace else (out, None)


def kernel(x, w1, b1, w2, b2):
    out, _ = run_moe(np.asarray(x), np.asarray(w1), np.asarray(b1),
                     np.asarray(w2), np.asarray(b2))
    return out



# revision 2
# speedup vs baseline: 1.1775x; 1.1775x over previous
"""Expert-parallel MoE FFN kernel for Trainium2 (Bass/Tile).

Problem: y[b,e,n,:] = gelu(x[b,e,n,:] @ w1[e] + b1[e]) @ w2[e] + b2[e]
Shapes:  x (2,8,2048,1024), w1 (8,1024,4096), b1 (8,4096),
         w2 (8,4096,1024), b2 (8,1024)  -> out (2,8,2048,1024) fp32.

Sharding: expert-parallel, one expert per NeuronCore (8 cores).  Each core
processes its expert's 4096 tokens through the full FFN locally; no
cross-core communication.

Strategy (v2, fused bf16):
 - Host pre-packs per-expert inputs: x is cast to bf16 and transposed to
   xT [D, T] (so the device does ZERO transposes - the PE only ever runs
   matmuls), w1/w2 are cast to bf16.  bf16 matmul rate on the PE equals
   fp32r (1 col/cycle) but halves SBUF/DMA footprints; end-to-end rel err
   ~3.5e-3 vs the 2e-2 gate (verified numerically).
 - Both weight matrices live in SBUF for the whole kernel (64 KB/partition
   each in bf16), so there is no weight streaming and no [H,T] activation
   staging to DRAM (the baseline's 128 MiB HBM roundtrip is gone).
 - Tokens are processed in 8 blocks of 512.  Per block: GEMM1 produces
   hT [128, 32 h-tiles, 512] bf16 in SBUF (gelu+b1 fused into the PSUM
   drain on the scalar engine), then GEMM2 consumes hT as the stationary
   operand against resident w2, draining y tiles via the vector engine.
   The PE instruction stream is one long dense matmul sequence - no phase
   boundaries, no HAM cool-downs, all DMA (x-block in, y out) hidden.
 - PSUM: GEMM1 uses 2x [128,512] banks, GEMM2 2x [128,1024] (4 banks),
   both double-buffered; 6 of 8 banks total.
"""

import numpy as np

import concourse.bass as bass
import concourse.mybir as mybir
import concourse.tile as tile
from concourse import bacc
from concourse.bass_utils import run_bass_kernel_spmd

import ml_dtypes

P = 128
F32 = mybir.dt.float32
BF16 = mybir.dt.bfloat16
NP_BF16 = ml_dtypes.bfloat16

# Full-size problem constants (hardcoded; the grading harness calls
# kernel(**inputs) with exactly these shapes).
B, E, N, D, H = 2, 8, 2048, 1024, 4096
T = B * N
N_CORES = 8

TB = 512              # token block (GEMM1 moving free dim)
NB = T // TB          # token blocks
ND = D // P           # d tiles (GEMM1 contraction / GEMM2 output chunks)
NH = H // P           # h tiles
NT_B = TB // P        # token subtiles per block (GEMM2 output rows)
DCH = 512             # GEMM2 moving chunk (one PSUM bank)
NDC = D // DCH
HC = 1024             # w1 h-slice DMA chunk
NHC = H // HC


def emit_ffn(tc, xT, w1, b1, w2, b2, y, use_b2):
    """xT:[D,T] bf16, w1:[D,H] bf16, b1:[H] f32, w2:[H,D] bf16, b2:[D] f32,
    y:[T,D] f32."""
    nc = tc.nc

    xT_r = xT.rearrange("(dt p) t -> p dt t", p=P)
    w2_r = w2.rearrange("(ht p) d -> p ht d", p=P)

    with (
        tc.tile_pool(name="const", bufs=1, side="right") as const_pool,
        tc.tile_pool(name="wres", bufs=1, side="left") as wres_pool,
        tc.tile_pool(name="xt", bufs=2, side="right") as xt_pool,
        tc.tile_pool(name="out", bufs=2, side="right") as out_pool,
        tc.tile_pool(name="ph", bufs=2, space="PSUM", side="left") as ph_pool,
        tc.tile_pool(name="po", bufs=2, space="PSUM", side="right") as po_pool,
    ):
        b1_sb = const_pool.tile([P, NH], F32)
        nc.sync.dma_start(b1_sb[:], b1.rearrange("(ht p) -> p ht", p=P))
        if use_b2:
            b2_sb = const_pool.tile([P, D], F32)
            nc.sync.dma_start(b2_sb[:], b2.unsqueeze(0).broadcast_to([P, D]))

        w1_sb = wres_pool.tile([P, ND, H], BF16, name="w1_sb")
        w2_sb = wres_pool.tile([P, NH, D], BF16, name="w2_sb")
        hT_sb = wres_pool.tile([P, NH, TB], BF16, name="hT_sb")

        # token-block x tiles (double buffered)
        xt_tiles = [None] * NB

        def load_xt(g):
            xt_tiles[g] = xt_pool.tile([P, ND, TB], BF16, name="xt")
            nc.sync.dma_start(xt_tiles[g][:],
                              xT_r[:, :, g * TB:(g + 1) * TB])

        # DMA emission order = ring order: first x block, then w1 in
        # h-slice-major order (so GEMM1 h-tile 0 is ready asap), then w2.
        load_xt(0)
        for hc in range(NHC):
            for dt in range(ND):
                nc.sync.dma_start(
                    w1_sb[:, dt, hc * HC:(hc + 1) * HC],
                    w1[dt * P:(dt + 1) * P, hc * HC:(hc + 1) * HC])
        for ht in range(NH):
            nc.sync.dma_start(w2_sb[:, ht, :], w2_r[:, ht, :])

        for g in range(NB):
            if g + 1 < NB:
                load_xt(g + 1)
            xt = xt_tiles[g]
            xt_tiles[g] = None

            # ---- GEMM1: hT[h,t] = gelu(sum_d w1[d,h]*xT[d,t] + b1[h]) ----
            with nc.named_scope(f"gemm1_b{g}"):
                for ht in range(NH):
                    psum_h = ph_pool.tile([P, TB], F32, name="psum_h")
                    for dt in range(ND):
                        nc.tensor.matmul(
                            psum_h[:],
                            w1_sb[:, dt, ht * P:(ht + 1) * P],
                            xt[:, dt, :],
                            start=(dt == 0), stop=(dt == ND - 1))
                    nc.scalar.activation(
                        hT_sb[:, ht, :], psum_h[:],
                        mybir.ActivationFunctionType.Gelu_apprx_tanh,
                        bias=b1_sb[:, ht:ht + 1], scale=1.0)

            # ---- GEMM2: y[t,d] = sum_h hT[h,t]*w2[h,d] (+ b2) ------------
            with nc.named_scope(f"gemm2_b{g}"):
                for tt in range(NT_B):
                    psum_o = po_pool.tile([P, D], F32, name="psum_o")
                    for ht in range(NH):
                        for dc in range(NDC):
                            nc.tensor.matmul(
                                psum_o[:, dc * DCH:(dc + 1) * DCH],
                                hT_sb[:, ht, tt * P:(tt + 1) * P],
                                w2_sb[:, ht, dc * DCH:(dc + 1) * DCH],
                                start=(ht == 0), stop=(ht == NH - 1))
                    out_sb = out_pool.tile([P, D], F32, name="out_sb")
                    if use_b2:
                        nc.vector.tensor_add(out_sb[:], psum_o[:], b2_sb[:])
                    else:
                        nc.vector.tensor_copy(out_sb[:], psum_o[:])
                    t0 = (g * NT_B + tt) * P
                    nc.sync.dma_start(y[t0:t0 + P, :], out_sb[:])


def build_module(use_b2=False):
    nc = bacc.Bacc(None, target_bir_lowering=False)
    xT = nc.dram_tensor("xT", [D, T], BF16, kind="ExternalInput")
    w1 = nc.dram_tensor("w1", [D, H], BF16, kind="ExternalInput")
    b1 = nc.dram_tensor("b1", [H], F32, kind="ExternalInput")
    w2 = nc.dram_tensor("w2", [H, D], BF16, kind="ExternalInput")
    b2 = (nc.dram_tensor("b2", [D], F32, kind="ExternalInput")
          if use_b2 else None)
    y = nc.dram_tensor("y", [T, D], F32, kind="ExternalOutput")

    with tile.TileContext(nc) as tc:
        emit_ffn(tc, xT[:], w1[:], b1[:], w2[:],
                 b2[:] if use_b2 else None, y[:], use_b2)
    nc.compile()
    return nc


_module_cache = {}


def _get_module(use_b2):
    if use_b2 not in _module_cache:
        _module_cache[use_b2] = build_module(use_b2=use_b2)
    return _module_cache[use_b2]


def run_moe(x, w1, b1, w2, b2, trace=False):
    """x:(B,E,N,D) w1:(E,D,H) b1:(E,H) w2:(E,H,D) b2:(E,D) -> (B,E,N,D)."""
    x = np.asarray(x)
    w1 = np.asarray(w1)
    b1 = np.asarray(b1)
    w2 = np.asarray(w2)
    b2 = np.asarray(b2)
    Bx, Ex, Nx, Dx = x.shape
    use_b2 = bool(np.any(b2))
    nc = _get_module(use_b2)

    # Host-side pack: bf16 cast everywhere, x transposed to [E, D, T] so
    # tokens are the free dim on device (no on-device transposes at all).
    xT = np.ascontiguousarray(
        x.astype(NP_BF16).transpose(1, 3, 0, 2).reshape(Ex, Dx, Bx * Nx))
    w1b = np.ascontiguousarray(w1.astype(NP_BF16))
    w2b = np.ascontiguousarray(w2.astype(NP_BF16))
    b1f = np.ascontiguousarray(b1.astype(np.float32))

    in_maps = []
    for e in range(Ex):
        m = {"xT": xT[e], "w1": w1b[e], "b1": b1f[e], "w2": w2b[e]}
        if use_b2:
            m["b2"] = np.ascontiguousarray(b2[e].astype(np.float32))
        in_maps.append(m)

    br = run_bass_kernel_spmd(nc, in_maps, core_ids=list(range(Ex)),
                              trace=trace)
    ys = np.stack([br.results[e]["y"] for e in range(Ex)], axis=0)  # [E,T,D]
    out = ys.reshape(Ex, Bx, Nx, Dx).reshape(Bx, Ex, Nx, Dx)
    return (out, br) if trace else (out, None)


def kernel(x, w1, b1, w2, b2):
    out, _ = run_moe(np.asarray(x), np.asarray(w1), np.asarray(b1),
                     np.asarray(w2), np.asarray(b2))
    return out
